# revision 61
# baseline (speedup 1.0000x reference)
"""Causal self-attention (B=2, T=2048, C=1024, H=16, D=64) on 8 trn2 NeuronCores.

Sharding: core c -> batch b = c // 4, head group g = c % 4 (heads 4g..4g+3).
Each core computes, for its batch and its 4 heads:
    qkT   = Wqk_local^T @ x_b^T          [512, 2048]   (q/k transposed layout)
    v     = x_b @ Wv_local               [2048, 256]
    sT    = k q^T (per head)             [k, q]; exp(s/8); causal tri mask
    pv    = (ones|v)^T @ exp(sT)         [128, q]: 64 denom rows + 64 attn rows
    at    = attn / denom                 (Pool partition-broadcast + divide)
    y_par = atT-contraction @ Wp_local   [2048, 1024]
Host: y[b] = sum of the 4 partials + b_proj + (b_attn_v @ W_proj).

The qkv projections (wave-1/w2 qkT slices and the v projection) run as
fp8e4 DoubleRow matmuls (0.5 cyc/col, 2x128-deep contraction slots per
instruction) with 3-term error compensation: x and 32*W are split hi/lo
on the HOST (hi = fp8(a), lo = fp8(a - hi)); each ck-pair issues
(wh,xh) (wl,xh) (wh,xl); the dropped xl*wl term is O(0.1%).  The x32
weight scaling keeps N(0,1/32^2) weights out of fp8 subnormals and is
undone for free: qk_sb holds 32*q / 32*k (EXP_SCALE absorbs the 1/1024),
vst holds 32*v, and the host divides W_proj by 32.  Measured: rel err
4.57e-3, BETTER than the all-bf16 baseline's 5.62e-3 (fp8 hi+lo carries
~7 mantissa bits vs bf16's 8, and the scores matmul is unchanged).
Scores / pv / out-proj must stay bf16: measured on HW, st-fp8 2.2e-2,
v-fp8 2.5e-2, scores-fp8 3.5e-2 against the 2e-2 gate.  DVE tensor
divide and base!=0 reciprocal_approx_fast do NOT work on HW (compile
error / garbage) - normalization needs the recip@base0 + mirror-DMA
dance.  All heads' pv layout is (ones|v): denom rows 0:64, attn rows
64:128; at_sb rows [odd|even] per pair, host permutes W_proj to match.

Normalization per chunk: two reciprocal_approx_fast at partition base 0
(base 64 is broken on HW), ONE SBUF->SBUF DMA mirrors both recips to
rows 64:128, two DVE muls, and the odd head's rows take a second DMA
down into at_sb[0:64].

Schedule: each attention iteration carries a filler quantum (v / wave-2
qkT / output projection matmuls) sized so PE stays ahead of the ACT exp
- exp throughput (~1.04us per [2,512] tile) is the binding cadence in
the big passes.  Pass order interleaves pairs chunk-wise ending P1C3;
QUANTA retuned for the cheaper fp8 fillers (TimelineSim-swept).  NOTE
for future tuning: reordering passes to end with P1C0 measurably LOSES
~6-9us because chunk-j proj fillers only unlock after BOTH pairs finish
chunk j - the interleaved order streams proj from pass 3 onward.
"""

import os
import sys

import numpy as np

try:
    import concourse.bass  # noqa: F401
except ImportError:
    for _p in ("/opt/trn_rl_repo", "/root/.axon_site/_ro/trn_rl_repo"):
        if os.path.isdir(_p) and _p not in sys.path:
            sys.path.insert(0, _p)

import concourse.bass as bass  # noqa: E402,F401
import concourse.mybir as mybir  # noqa: E402
import concourse.tile as tile  # noqa: E402
from concourse import bacc  # noqa: E402
from concourse.bass_utils import run_bass_kernel_spmd  # noqa: E402

B, T, C, H, D = 2, 2048, 1024, 16, 64
# K2_SUBNORM!=0 would normalize the last chunk in sub-ranges while the
# same psum bank still accumulates later columns.  CoreSim rejects that,
# and hardware PROVES it wrong (rel err 0.1): psum reads during an open
# accumulation group return corrupt data.  Keep 0.
SUBNORM = int(os.environ.get("K2_SUBNORM", "0"))
HL = 4          # heads per core
N_CORES = 8
QCH = 512       # q-chunk width (one PSUM bank of fp32)
NKT = T // 128  # 16 k-tiles per head
NQC = T // QCH  # 4 q-chunks

F32 = mybir.dt.float32
BF16 = mybir.dt.bfloat16
F8 = mybir.dt.float8e4
MMDT = BF16
DR = mybir.MatmulPerfMode.DoubleRow
# fp8 hi/lo weight scaling: W quantized at 32x (keeps N(0,1/32^2) weights
# out of fp8 subnormals); undone via exp-scale (qk) and host W_proj/32 (v)
WSC = 32.0
EXP_SCALE = 0.125 / (WSC * WSC)

LAST_RESULT = None

# (pair, chunk) pass order; P1C0 last (small tail, its proj drains last).
# P0C3 before P1C2 so the proj groups each pair-completed chunk unlocks
# land in the pass AFTER next - P1C3 then has fillers of its own.
PASSES = [(0, 0), (1, 0), (0, 1), (1, 1), (0, 2), (1, 2), (0, 3), (1, 3)]


def _body(tc):
    nc = tc.nc
    ACT = mybir.ActivationFunctionType
    ALU = mybir.AluOpType

    xh = nc.dram_tensor("xh", [C, T], F8, kind="ExternalInput").ap()
    xl = nc.dram_tensor("xl", [C, T], F8, kind="ExternalInput").ap()
    wqh = nc.dram_tensor("wqh", [C, 512], F8, kind="ExternalInput").ap()
    wql = nc.dram_tensor("wql", [C, 512], F8, kind="ExternalInput").ap()
    wvh = nc.dram_tensor("wvh", [C, 256], F8, kind="ExternalInput").ap()
    wvl = nc.dram_tensor("wvl", [C, 256], F8, kind="ExternalInput").ap()
    wp = nc.dram_tensor("wp", [256, C], MMDT, kind="ExternalInput").ap()
    bqk = nc.dram_tensor("bqk", [128, 4], F32, kind="ExternalInput").ap()
    tri = nc.dram_tensor("tri", [128, 128], MMDT, kind="ExternalInput").ap()
    y = nc.dram_tensor("y", [T, C], MMDT, kind="ExternalOutput").ap()

    # ---------------- persistent SBUF ----------------
    persist = tc.alloc_tile_pool(name="persist", bufs=1)
    qk_sb = persist.tile([128, 2, 2, T], MMDT, tag="qk")    # [p, hpair, q/k, t]
    vst = persist.tile([128, NKT, HL, 128], MMDT, tag="vst")  # per-head ones|v
    at_sb = persist.tile([128, 2, T], MMDT, tag="at")       # [oddattn|evenattn]
    wp_sb = persist.tile([128, 2, C], MMDT, tag="wp")
    xh_sb = persist.tile([128, 8, T], F8, tag="xh")
    xl_sb = persist.tile([128, 8, T], F8, tag="xl")
    wqh_sb = persist.tile([128, 8, 512], F8, tag="wqh")
    wql_sb = persist.tile([128, 8, 512], F8, tag="wql")
    wvh_sb = persist.tile([128, 8, 256], F8, tag="wvh")
    wvl_sb = persist.tile([128, 8, 256], F8, tag="wvl")
    bqk_sb = persist.tile([128, 4], F32, tag="bqk")
    tri_sb = persist.tile([128, 1, 128], MMDT, tag="tri")

    # small constants + norm-internal DMAs ride the gpsimd SWDGE queue:
    # verified compilable, and it keeps the SP/HWDGE path clear
    nc.gpsimd.dma_start(out=tri_sb[:, 0, :], in_=tri)
    nc.gpsimd.dma_start(out=bqk_sb, in_=bqk)

    # PE p-state warmup: the tensor engine ramps 0.65->1.2->2.4 GHz over
    # ~3us of continuous work.  Burn the initial DMA wait on junk matmuls
    # (zeroed operands) so wave 1 runs at full clock.
    warm_ps_pool = tc.alloc_tile_pool(name="warm_ps", bufs=1, space="PSUM")
    warm = persist.tile([128, QCH], MMDT, tag="warm")
    warm_ps = warm_ps_pool.tile([128, QCH], F32, tag="warmps")
    nc.vector.memzero(warm)
    for _ in range(int(os.environ.get('K2_WARM', '6'))):
        nc.tensor.matmul(warm_ps, lhsT=warm[:, 0:128], rhs=warm,
                         start=True, stop=True)
    warm_ps_pool.release()

    # ones half of vst for every head: (ones|v) -> denominator rows 0:64
    onesrc = tri_sb[:, 0, 127:128]
    nc.vector.tensor_copy(
        out=vst[:, :, :, 0:64],
        in_=onesrc.broadcast_to([128, NKT, HL, 64]))

    # ---------------- input DMAs (SP queue; few, large) ----------------
    xh_r = xh.rearrange("(c p) t -> p c t", p=128)
    xl_r = xl.rearrange("(c p) t -> p c t", p=128)
    wqh_r = wqh.rearrange("(c p) n -> p c n", p=128)
    wql_r = wql.rearrange("(c p) n -> p c n", p=128)
    # wave-1 feed in 2-ck waves; term order (wh,xh) (wl,xh) (wh,xl)
    # matches arrival: xh+wqh waves, then wql (small), then xl
    for ck in range(0, 8, 2):
        nc.sync.dma_start(out=xh_sb[:, ck:ck + 2, 0:QCH],
                          in_=xh_r[:, ck:ck + 2, 0:QCH])
        nc.sync.dma_start(out=wqh_sb[:, ck:ck + 2, 0:256],
                          in_=wqh_r[:, ck:ck + 2, 0:256])
    nc.sync.dma_start(out=wql_sb[:, :, 0:256], in_=wql_r[:, :, 0:256])
    nc.sync.dma_start(out=xl_sb[:, :, 0:QCH], in_=xl_r[:, :, 0:QCH])
    nc.sync.dma_start(out=wvh_sb,
                      in_=wvh.rearrange("(c p) n -> p c n", p=128))
    nc.sync.dma_start(out=wvl_sb,
                      in_=wvl.rearrange("(c p) n -> p c n", p=128))
    nc.sync.dma_start(out=wqh_sb[:, :, 256:512], in_=wqh_r[:, :, 256:512])
    nc.sync.dma_start(out=wql_sb[:, :, 256:512], in_=wql_r[:, :, 256:512])
    for qq in (1, 2, 3):
        nc.sync.dma_start(out=xh_sb[:, :, qq * QCH:(qq + 1) * QCH],
                          in_=xh_r[:, :, qq * QCH:(qq + 1) * QCH])
        nc.sync.dma_start(out=xl_sb[:, :, qq * QCH:(qq + 1) * QCH],
                          in_=xl_r[:, :, qq * QCH:(qq + 1) * QCH])
    nc.sync.dma_start(out=wp_sb, in_=wp.rearrange("(c p) n -> p c n", p=128))

    # ---------------- pools ----------------
    # PSUM budget (8 banks): ps_s double-buffered [128,2,512] = 4,
    # pv 3 x [128,512] = 3, filler 1 x [128,512] = 1.
    pss_pool = tc.alloc_tile_pool(name="ps_s", bufs=2, space="PSUM")
    pv_pool = tc.alloc_tile_pool(name="ps_pv", bufs=1, space="PSUM")
    fill_pool = tc.alloc_tile_pool(name="ps_fill", bufs=1, space="PSUM")
    st_pool = tc.alloc_tile_pool(
        name="st", bufs=int(os.environ.get("K2_STBUFS", "13")))
    pvsb_pool = tc.alloc_tile_pool(name="pvsb", bufs=3)
    dmap_pool = tc.alloc_tile_pool(name="dmap", bufs=2)
    yo_pool = tc.alloc_tile_pool(name="yo", bufs=6)

    SLICE_MAP = {0: (0, 0), 1: (0, 1), 2: (1, 0), 3: (1, 1)}  # s -> (hp, qk)
    _pvc = [0]

    def pv_tile(name):
        _pvc[0] += 1
        return pv_pool.tile([128, QCH], F32, tag=f"pv{_pvc[0] % 3}", name=name)

    _fc = [0]
    state = {"task": None, "mmdone": 0, "tile": None, "drain": False}

    def fill_tile(name):
        _fc[0] += 1
        if state["drain"]:
            # attention over: rotate through the freed pv banks too
            return pv_tile(name)
        return fill_pool.tile([128, QCH], F32, tag="fl0", name=name)

    # ---------------- filler task machinery ----------------
    # All filler tasks produce a [128,256] psum tile (half a bank) so two
    # tasks pipeline through one bank.  kinds:
    #   ("v", kt)         8 mm + evac (v projection, one k-tile)
    #   ("w2", s, tch, h) 8 mm + evac (qkT slice, 256-col half)
    #   ("proj", tt, n2, q) 2 mm + evac (output projection 256-col quarter)
    tasks = []          # ordered pending tasks
    proj_ready = []     # proj tasks appended as chunks complete
    yhalf = {}          # tt -> [yt tile, done-marker set]

    def task_mm_count(t):
        return {"v": 12, "w2": 12, "proj": 2, "warm": 4}[t[0]]

    # fp8 3-term compensation: terms (w_hi,x_hi) (w_lo,x_hi) (w_hi,x_lo);
    # each k step is one DoubleRow over a ck-pair
    def emit_task_mm(t, k):
        kind = t[0]
        term, pr = divmod(k, 4)
        if kind == "warm":
            nc.tensor.matmul(
                state["tile"], lhsT=warm[:, 0:128], rhs=warm,
                start=(k == 0), stop=(k == 3),
            )
        elif kind == "v":
            kt = t[1]
            xs = (xh_sb, xh_sb, xl_sb)[term]
            ws = (wvh_sb, wvl_sb, wvh_sb)[term]
            nc.tensor.matmul(
                state["tile"][:, 0:256],
                lhsT=xs[:, 2 * pr:2 * pr + 2, kt * 128:(kt + 1) * 128],
                rhs=ws[:, 2 * pr:2 * pr + 2, :],
                start=(k == 0), stop=(k == 11), perf_mode=DR,
            )
        elif kind == "w2":
            s, tch = t[1], t[2]
            ws = (wqh_sb, wql_sb, wqh_sb)[term]
            xs = (xh_sb, xh_sb, xl_sb)[term]
            nc.tensor.matmul(
                state["tile"],
                lhsT=ws[:, 2 * pr:2 * pr + 2, s * 128:(s + 1) * 128],
                rhs=xs[:, 2 * pr:2 * pr + 2, tch * QCH:(tch + 1) * QCH],
                start=(k == 0), stop=(k == 11), perf_mode=DR,
            )
        else:
            tt, n2 = t[1], t[2]
            nc.tensor.matmul(
                state["tile"],
                lhsT=at_sb[:, k, tt * 128:(tt + 1) * 128],
                rhs=wp_sb[:, k, n2 * QCH:(n2 + 1) * QCH],
                start=(k == 0), stop=(k == 1),
            )

    def evac_task(t, yt_engine=None):
        kind = t[0]
        p = state["tile"]
        use_act = ((_fc[0] % 2 == 0) and not state["drain"]
                   and state.get("actok", True))
        if kind == "warm":
            return
        if kind == "v":
            kt = t[1]
            o_ = vst[:, kt, :, 64:128]
            i_ = p[:, 0:256].rearrange("p (h d) -> p h d", h=HL)
            if use_act:
                nc.scalar.activation(out=o_, in_=i_, func=ACT.Copy)
            else:
                nc.vector.tensor_copy(out=o_, in_=i_)
        elif kind == "w2":
            s, tch = t[1], t[2]
            hp, qk = SLICE_MAP[s]
            if use_act:
                nc.scalar.activation(
                    out=qk_sb[:, hp, qk, tch * QCH:(tch + 1) * QCH],
                    in_=p, func=ACT.Identity, bias=bqk_sb[:, s:s + 1],
                )
            else:
                nc.vector.tensor_scalar_add(
                    out=qk_sb[:, hp, qk, tch * QCH:(tch + 1) * QCH],
                    in0=p, scalar1=bqk_sb[:, s:s + 1],
                )
        else:
            tt, n2 = t[1], t[2]
            if tt not in yhalf:
                yhalf[tt] = [yo_pool.tile([128, C], MMDT, tag="yt",
                                          name=f"yt{tt}"), 0, set(), set()]
            yt = yhalf[tt][0]
            # alternate DVE/ACT for tail evacs so they pipeline
            eng = yt_engine if (yt_engine is not None and n2 == 1) else nc.vector
            if eng is nc.scalar:
                nc.scalar.activation(
                    out=yt[:, n2 * QCH:(n2 + 1) * QCH], in_=p, func=ACT.Copy)
            else:
                nc.vector.tensor_copy(
                    out=yt[:, n2 * QCH:(n2 + 1) * QCH], in_=p)
            yhalf[tt][1] += 1
            yhalf[tt][2].add(n2)
            if state["drain"] and os.environ.get("K2_DFULL", "1") == "1":
                if yhalf[tt][1] == 2:
                    nc.sync.dma_start(
                        out=y[tt * 128:(tt + 1) * 128, :], in_=yt)
                    del yhalf[tt]
            elif state["drain"]:
                # stream out every evac'd-but-unsent half; a half evac'd
                # pre-drain must not be stranded when its partner lands
                for h in sorted(yhalf[tt][2] - yhalf[tt][3]):
                    q = nc.sync if h == 0 else nc.gpsimd
                    q.dma_start(
                        out=y[tt * 128:(tt + 1) * 128,
                              h * QCH:(h + 1) * QCH],
                        in_=yt[:, h * QCH:(h + 1) * QCH])
                    yhalf[tt][3].add(h)
                if yhalf[tt][1] == 2:
                    del yhalf[tt]
            elif yhalf[tt][1] == 2:
                yq = nc.gpsimd if os.environ.get("K2_YQ", "sp") == "gp" \
                    else nc.sync
                yq.dma_start(
                    out=y[tt * 128:(tt + 1) * 128, :], in_=yt)
                del yhalf[tt]

    def emit_quantum(max_ns, yt_engine=None):
        """Emit filler matmuls worth up to ~max_ns of PE time."""
        ns = 0
        while ns < max_ns:
            if state["task"] is None:
                src = tasks if tasks else proj_ready
                if not src:
                    return ns
                state["task"] = src.pop(0)
                state["mmdone"] = 0
                kind = state["task"][0]
                state["tile"] = fill_tile(f"f_{kind}")
            t = state["task"]
            n = task_mm_count(t)
            k = state["mmdone"]
            emit_task_mm(t, k)
            ns += {"v": 53, "w2": 107, "proj": 213, "warm": 213}[t[0]]
            state["mmdone"] += 1
            if state["mmdone"] == n:
                evac_task(t, yt_engine=yt_engine)
                state["task"] = None
        return ns

    # ---------------- wave 1: qkT slices s0, s1 chunk 0 ----------------
    ps_w1 = {s: pv_tile(f"w1_{s}") for s in (0, 1)}
    wmw = int(os.environ.get("K2_WMW", "4"))
    for k in range(12):
        term, pr = divmod(k, 4)
        if k in (4, 8) and wmw:
            wm = fill_pool.tile([128, QCH], F32, tag="fl0", name=f"wm{k}")
            for wi in range(wmw):
                nc.tensor.matmul(wm, lhsT=warm[:, 0:128], rhs=warm,
                                 start=(wi == 0), stop=(wi == wmw - 1))
        ws = (wqh_sb, wql_sb, wqh_sb)[term]
        xs = (xh_sb, xh_sb, xl_sb)[term]
        for s in (0, 1):
            nc.tensor.matmul(
                ps_w1[s],
                lhsT=ws[:, 2 * pr:2 * pr + 2, s * 128:(s + 1) * 128],
                rhs=xs[:, 2 * pr:2 * pr + 2, 0:QCH],
                start=(k == 0), stop=(k == 11), perf_mode=DR,
            )
    tasks.extend([("v", 0)])
    for s in (0, 1):
        hp, qk = SLICE_MAP[s]
        nc.vector.tensor_scalar_add(
            out=qk_sb[:, hp, qk, 0:QCH],
            in0=ps_w1[s], scalar1=bqk_sb[:, s:s + 1],
        )

    # filler stream in deadline order
    def w2t(s, tch):
        return [("w2", s, tch)]

    tasks.extend([("v", 1), ("v", 2), ("v", 3)]
                 + w2t(2, 0) + w2t(3, 0)
                 + w2t(0, 1) + w2t(1, 1)
                 + [("v", 4), ("v", 5), ("v", 6), ("v", 7)]
                 + w2t(2, 1) + w2t(3, 1)
                 + w2t(0, 2) + w2t(1, 2)
                 + [("v", 8), ("v", 9), ("v", 10), ("v", 11)]
                 + w2t(2, 2) + w2t(3, 2)
                 + w2t(0, 3) + w2t(1, 3)
                 + [("v", 12), ("v", 13), ("v", 14), ("v", 15)]
                 + w2t(2, 3) + w2t(3, 3))

    # hard prerequisites per pass (drained before the pass starts):
    # number of leading tasks that must be complete
    prereq = {
        (1, 0): w2t(2, 0) + w2t(3, 0),
        (0, 1): w2t(0, 1),
        (1, 1): w2t(2, 1) + w2t(3, 1),
        (0, 2): w2t(0, 2),
        (1, 2): w2t(2, 2) + w2t(3, 2),
        (0, 3): w2t(0, 3),
        (1, 3): w2t(2, 3) + w2t(3, 3),
    }
    # v deadlines: (pass, i) -> v kt that must be finished by then handled
    # by quantum pacing; w2 k-slice deadlines likewise (s1tj needed at i=4j).

    def ensure_done(items):
        for it in items:
            while it in tasks or state["task"] == it:
                emit_quantum(1)

    # ---------------- attention ----------------
    done_chunks = {0: set(), 1: set()}

    def norm_first(hp, j, pv_e, pv_o, c0, c1):
        """Reciprocals + mirror DMA only (launch early, overlap)."""
        w = c1 - c0
        rc = pvsb_pool.tile([128, 2, QCH], F32, tag="pvsb",
                            name=f"rcf{hp}{j}{c0}")
        nc.vector.reciprocal_approx_fast(
            out=rc[0:64, 0, 0:w], in_=pv_o[0:64, c0:c1])
        nc.vector.reciprocal_approx_fast(
            out=rc[0:64, 1, 0:w], in_=pv_e[0:64, c0:c1])
        nq = nc.gpsimd if os.environ.get("K2_NQ", "sp") == "gp" else nc.sync
        nq.dma_start(out=rc[64:128, :, 0:w], in_=rc[0:64, :, 0:w])
        return rc

    def norm_second(hp, j, pv_e, pv_o, c0, c1, rc):
        """Normalize muls + odd-head move using a prepared rc tile."""
        w = c1 - c0
        js = slice(j * QCH + c0, j * QCH + c1)
        nc.vector.tensor_mul(
            out=at_sb[64:128, hp, js],
            in0=pv_e[64:128, c0:c1], in1=rc[64:128, 1, 0:w])
        am = dmap_pool.tile([128, QCH], MMDT, tag="dm",
                            name=f"am{hp}{j}{c0}")
        nc.vector.tensor_mul(
            out=am[64:128, 0:w],
            in0=pv_o[64:128, c0:c1], in1=rc[64:128, 0, 0:w])
        nq = nc.gpsimd if os.environ.get("K2_NQ", "sp") == "gp" else nc.sync
        nq.dma_start(out=at_sb[0:64, hp, js], in_=am[64:128, 0:w])

    def norm_chunk(hp, j, pv_e, pv_o, c0, c1):
        """Normalize columns [c0:c1) of chunk j of pair hp.

        Both heads' psum: denominator rows 0:64, attn rows 64:128.
        at_sb rows: 0:64 = odd head, 64:128 = even head (wp matches).
        reciprocal_approx_fast only runs at partition base 0, and DVE
        cannot move partitions, so both reciprocals are computed in the
        low half and ONE SBUF->SBUF DMA mirrors them to the high half;
        the odd head's normalized rows take a second small DMA down into
        at_sb's low half.
        """
        w = c1 - c0
        js = slice(j * QCH + c0, j * QCH + c1)
        rc = pvsb_pool.tile([128, 2, QCH], F32, tag="pvsb",
                            name=f"rc{hp}{j}")
        nq = nc.gpsimd if os.environ.get("K2_NQ", "sp") == "gp" else nc.sync
        nc.vector.reciprocal_approx_fast(
            out=rc[0:64, 1, 0:w], in_=pv_e[0:64, c0:c1])
        if os.environ.get("K2_NSPLIT", "0") == "1":
            nq.dma_start(out=rc[64:128, 1, 0:w], in_=rc[0:64, 1, 0:w])
            nc.vector.reciprocal_approx_fast(
                out=rc[0:64, 0, 0:w], in_=pv_o[0:64, c0:c1])
            nq.dma_start(out=rc[64:128, 0, 0:w], in_=rc[0:64, 0, 0:w])
        else:
            nc.vector.reciprocal_approx_fast(
                out=rc[0:64, 0, 0:w], in_=pv_o[0:64, c0:c1])
            nq.dma_start(out=rc[64:128, :, 0:w], in_=rc[0:64, :, 0:w])
        nc.vector.tensor_mul(
            out=at_sb[64:128, hp, js],
            in0=pv_e[64:128, c0:c1], in1=rc[64:128, 1, 0:w])
        am = dmap_pool.tile([128, QCH], MMDT, tag="dm", name=f"am{hp}{j}")
        nc.vector.tensor_mul(
            out=am[64:128, 0:w],
            in0=pv_o[64:128, c0:c1], in1=rc[64:128, 0, 0:w])
        nq.dma_start(out=at_sb[0:64, hp, js], in_=am[64:128, 0:w])

    # per-pass filler quantum (ns of PE time per iteration): generous in
    # the early passes where the exp duty is low, lean late where the
    # proj stream is the only filler left
    QUANTA = [int(q) for q in os.environ.get(
        "K2_QUANTA", "1100,600,500,200,300,150,150,300").split(",")]
    actok_max = int(os.environ.get("K2_ACTOK", "1"))
    for pidx, (hp, j) in enumerate(PASSES):
        state["actok"] = pidx < actok_max
        ensure_done(prereq.get((hp, j), []))
        last_pass = (hp, j) == PASSES[-1]
        quantum = QUANTA[pidx]
        pv_e = pv_tile(f"pv{hp}_{j}e")
        pv_o = pv_tile(f"pv{hp}_{j}o")
        niter = 4 * j + 4
        yt_eng = nc.scalar if os.environ.get("K2_YACT", "0") == "1" \
            or last_pass else None
        pend = None   # (i, lo, st) pv matmuls deferred one iteration

        def emit_pv(p):
            i, lo, st = p
            nc.tensor.matmul(
                pv_e[:, lo:], lhsT=vst[:, i, 2 * hp, :],
                rhs=st[:, 0, lo:],
                start=(i == 0), stop=(i == niter - 1),
            )
            nc.tensor.matmul(
                pv_o[:, lo:], lhsT=vst[:, i, 2 * hp + 1, :],
                rhs=st[:, 1, lo:],
                start=(i == 0), stop=(i == niter - 1),
            )
            if last_pass and SUBNORM == 1 and i >= 4 * j:
                # cols [128*s : 128*(s+1)] are final once k-tile i = 4j+s
                # is accumulated: norm slab-wise so proj streams in-pass
                s = i - 4 * j
                norm_chunk(hp, j, pv_e, pv_o, 128 * s, 128 * (s + 1))
                proj_ready.extend(
                    ("proj", 4 * j + s, n2) for n2 in range(2))
            elif last_pass and SUBNORM == 2 and i == 4 * j + 1:
                # cols 0:256 are final after k-tile 4j+1: norm the first
                # half early so its proj overlaps the last two iterations
                norm_chunk(hp, j, pv_e, pv_o, 0, 256)
                proj_ready.extend(
                    ("proj", tt, n2) for tt in (4 * j, 4 * j + 1)
                    for n2 in range(2))

        for i in range(niter):
            j0 = i // 4
            lo = i * 128 - j0 * QCH if j == j0 else 0
            ps_s = pss_pool.tile([128, 2, QCH], F32, tag="pss", name="pss")
            st = st_pool.tile([128, 2, QCH], MMDT, tag="st", name="st")
            if pend is not None and os.environ.get("K2_PVF", "0") == "1":
                emit_pv(pend)
                pend = None
            for b, off in ((0, 0), (1, 64)):
                nc.tensor.matmul(
                    ps_s[:, b, lo:],
                    lhsT=qk_sb[off:off + 64, hp, 1, i * 128:(i + 1) * 128],
                    rhs=qk_sb[off:off + 64, hp, 0, j * QCH + lo:(j + 1) * QCH],
                    start=True, stop=True,
                )
            nc.scalar.activation(
                out=st[:, :, lo:], in_=ps_s[:, :, lo:],
                func=ACT.Exp, scale=EXP_SCALE,
            )
            if j == j0:
                nc.vector.tensor_mul(
                    out=st[:, :, lo:lo + 128],
                    in0=st[:, :, lo:lo + 128],
                    in1=tri_sb.broadcast_to([128, 2, 128]),
                )
            # filler quantum fills the exp latency; pv lags one iteration
            emit_quantum(quantum, yt_engine=yt_eng)
            if pend is not None:
                emit_pv(pend)
            pend = (i, lo, st)
        emit_quantum(int(os.environ.get("K2_PQ", "700")), yt_engine=yt_eng)
        emit_pv(pend)
        if last_pass:
            if SUBNORM == 0:
                norm_chunk(hp, j, pv_e, pv_o, 0, QCH)
                proj_ready.extend(
                    ("proj", tt, n2) for tt in range(4 * j, 4 * j + 4)
                    for n2 in range(2))
            elif SUBNORM == 2:
                norm_chunk(hp, j, pv_e, pv_o, 256, QCH)
                proj_ready.extend(
                    ("proj", tt, n2) for tt in (4 * j + 2, 4 * j + 3)
                    for n2 in range(2))
        else:
            norm_chunk(hp, j, pv_e, pv_o, 0, 512)
            done_chunks[hp].add(j)
            if j in done_chunks[1 - hp]:
                proj_ready.extend(
                    ("proj", tt, n2) for tt in range(4 * j, 4 * j + 4)
                    for n2 in range(2))

    # ---------------- drain ----------------
    state["drain"] = True
    while tasks or proj_ready or state["task"] is not None:
        emit_quantum(10 ** 9, yt_engine=nc.scalar)

    for p in (yo_pool, dmap_pool, pvsb_pool, st_pool, fill_pool, pv_pool,
              pss_pool, persist):
        p.release()


_PROGRAM = None


def build_program():
    global _PROGRAM
    if _PROGRAM is None:
        nc = bacc.Bacc("TRN2", debug=False, num_devices=N_CORES)
        with tile.TileContext(nc) as tc:
            _body(tc)
        nc.compile()
        _PROGRAM = nc
    return _PROGRAM


def _bf16(a):
    import ml_dtypes
    return np.asarray(a, np.float32).astype(ml_dtypes.bfloat16)


def _f8_hilo(a):
    """fp8e4 hi/lo split: hi = fp8(a), lo = fp8(a - hi)."""
    import ml_dtypes
    a = np.asarray(a, np.float32)
    hi = a.astype(ml_dtypes.float8_e4m3)
    lo = (a - hi.astype(np.float32)).astype(ml_dtypes.float8_e4m3)
    return hi, lo


def make_in_maps(x, W_attn, b_attn, W_proj):
    """Host-side shard: per-core input dict."""
    x = np.asarray(x, np.float32)
    W_attn = np.asarray(W_attn, np.float32)
    b_attn = np.asarray(b_attn, np.float32)
    W_proj = np.asarray(W_proj, np.float32)
    tri = np.triu(np.ones((128, 128), np.float32))  # tri[k, q] = k <= q
    in_maps = []
    for c in range(N_CORES):
        b, g = divmod(c, 4)
        xth, xtl = _f8_hilo(x[b].T)  # [C, T]
        q0 = 256 * g
        # [q01 | k01 | q23 | k23]
        cols = np.r_[q0:q0 + 128, C + q0:C + q0 + 128,
                     q0 + 128:q0 + 256, C + q0 + 128:C + q0 + 256]
        wqh_, wql_ = _f8_hilo(W_attn[:, cols] * WSC)  # [C, 512]
        wvh_, wvl_ = _f8_hilo(W_attn[:, 2 * C + q0:2 * C + q0 + 256] * WSC)
        # at_sb rows per pair = [odd head | even head] -> permute wp rows
        rows = np.r_[q0 + 64:q0 + 128, q0:q0 + 64,
                     q0 + 192:q0 + 256, q0 + 128:q0 + 192]
        wp_l = np.ascontiguousarray(W_proj[rows, :] / WSC)
        bqk = np.ascontiguousarray(
            b_attn[cols].reshape(4, 128).T * WSC)  # [128, 4]
        in_maps.append({
            "xh": xth, "xl": xtl,
            "wqh": wqh_, "wql": wql_,
            "wvh": wvh_, "wvl": wvl_,
            "wp": _bf16(wp_l),
            "bqk": bqk, "tri": _bf16(tri),
        })
    return in_maps


def kernel(x, W_attn, b_attn, W_proj, b_proj):
    global LAST_RESULT
    W_attn = np.asarray(W_attn, np.float32)
    W_proj = np.asarray(W_proj, np.float32)
    b_attn = np.asarray(b_attn, np.float32)
    b_proj = np.asarray(b_proj, np.float32)

    nc = build_program()
    in_maps = make_in_maps(x, W_attn, b_attn, W_proj)
    res = run_bass_kernel_spmd(nc, in_maps, core_ids=list(range(N_CORES)))
    LAST_RESULT = res
    parts = [np.asarray(r["y"], np.float32) for r in res.results]
    yb = [parts[0] + parts[1] + parts[2] + parts[3],
          parts[4] + parts[5] + parts[6] + parts[7]]
    out = np.stack(yb, axis=0)  # [B, T, C]
    out += (b_proj + b_attn[2 * C:] @ W_proj)[None, None, :]
    return out.astype(np.float32)

